# revision 5
# baseline (speedup 1.0000x reference)
# Trainium2 Bass kernel for DeeProBot MoE (top-2 of 8 experts, dense compute).
#
# Distribution: data-parallel over batch across 8 NeuronCores (sharding per
# spec hint); expert weights replicated. Aux-loss terms (importance/load) are
# reduced to [128, 8] partials per core on device; the final 128+8-element
# reduction and cv^2 arithmetic happen on host (trivial).
#
# Per-core pipeline (4096 rows, 8 tiles of 512 rows):
#   - DMA x tile (f32, natural layout)
#   - PE-transpose -> xT (f32 for exact gating, bf16 copy for expert matmuls)
#   - gating: logitsT = w_gate^T xT in f32 (layout A), PE-transpose back to
#     token-major, top-2 via DVE max8, gates/masks via DVE
#   - experts (dense): hT = relu(W1^T xT + b1) (bf16), z = h W2 (token-major),
#     ez = exp(z) on ACT with fused row-sum accumulation
#   - combine: y = sum_e (gate_e / s_e) * ez_e  (DVE fused scalar_tensor_tensor)
#   - head: out[:, o] = rowsum(y * Wout[:, o]) via DVE accum (OUT = 2)
import os
import sys

if "jax" not in sys.modules:
    # The PJRT execution path needs the axon platform; a leftover
    # JAX_PLATFORMS=cpu (used when running jax references) would hide it.
    os.environ.pop("JAX_PLATFORMS", None)

import numpy as np
import ml_dtypes

import concourse.bass as bass
import concourse.mybir as mybir
from concourse import bacc
from concourse.tile import TileContext
from concourse.bass_utils import run_bass_kernel_spmd
from concourse.masks import make_identity

B, IN, E, H, MO, OUT, TOPK = 32768, 512, 8, 128, 256, 2, 2
NCORES = 8
R = B // NCORES          # 4096 rows per core
TILE = 512               # rows per pipeline tile
NT = R // TILE           # 8 tiles
NCH = TILE // 128        # 4 chunks of 128 rows per tile
KC = IN // 128           # 4 contraction chunks

F32 = mybir.dt.float32
BF16 = mybir.dt.bfloat16
ALU = mybir.AluOpType
ACTF = mybir.ActivationFunctionType

_CACHED_NC = None


def _build_nc() -> bass.Bass:
    nc = bacc.Bacc()
    x_d = nc.declare_dram_parameter("x", [R, IN], F32, isOutput=False)
    w1_d = nc.declare_dram_parameter("w1sb", [128, E * KC * H], BF16, isOutput=False)
    w2_d = nc.declare_dram_parameter("w2sb", [128, E * MO], BF16, isOutput=False)
    wg_d = nc.declare_dram_parameter("wgsb", [128, KC * E], F32, isOutput=False)
    b1_d = nc.declare_dram_parameter("b1sb", [128, E], F32, isOutput=False)
    wo_d = nc.declare_dram_parameter("wosb", [128, OUT * MO], BF16, isOutput=False)
    out_d = nc.declare_dram_parameter("out", [R, OUT], F32, isOutput=True)
    st_d = nc.declare_dram_parameter("stats", [128, 2 * E], F32, isOutput=True)

    with TileContext(nc) as tc:
        with (
            tc.tile_pool(name="const", bufs=1) as cpool,
            tc.tile_pool(name="xio", bufs=2) as xpool,
            tc.tile_pool(name="work", bufs=2) as wpool,
            tc.tile_pool(name="small", bufs=2) as spool,
            tc.tile_pool(name="ezp", bufs=2) as ezpool,
            tc.tile_pool(name="ps_xt", bufs=2, space="PSUM") as ps_xt,
            tc.tile_pool(name="ps_lg", bufs=1, space="PSUM") as ps_lg,
            tc.tile_pool(name="ps_h", bufs=2, space="PSUM") as ps_h,
            tc.tile_pool(name="ps_z", bufs=2, space="PSUM") as ps_z,
        ):
            ident = cpool.tile([128, 128], F32)
            make_identity(nc, ident)
            w1_sb = cpool.tile([128, E * KC * H], BF16)
            nc.sync.dma_start(w1_sb, w1_d[:, :])
            w2_sb = cpool.tile([128, E * MO], BF16)
            nc.sync.dma_start(w2_sb, w2_d[:, :])
            wg_sb = cpool.tile([128, KC * E], F32)
            nc.sync.dma_start(wg_sb, wg_d[:, :])
            b1_sb = cpool.tile([128, E], F32)
            nc.sync.dma_start(b1_sb, b1_d[:, :])
            wo_sb = cpool.tile([128, OUT * MO], BF16)
            nc.sync.dma_start(wo_sb, wo_d[:, :])

            gacc = cpool.tile([128, E], F32)
            nc.vector.memset(gacc, 0.0)
            lacc = cpool.tile([128, E], F32)
            nc.vector.memset(lacc, 0.0)
            out_sb = cpool.tile([128, NT * NCH * OUT], F32)

            for t in range(NT):
                r0 = t * TILE
                # ---- load x tile (4 chunks of 128 rows)
                xc = xpool.tile([128, NCH, IN], F32, tag="xc")
                for c in range(NCH):
                    nc.sync.dma_start(
                        xc[:, c, :], x_d[r0 + c * 128 : r0 + (c + 1) * 128, :]
                    )
                # ---- transpose to xT (f32) and cast copy (bf16)
                xt_f = xpool.tile([128, KC, TILE], F32, tag="xtf")
                for kc in range(KC):
                    pst = ps_xt.tile([128, TILE], F32, tag="pst")
                    for c in range(NCH):
                        nc.tensor.transpose(
                            pst[:, c * 128 : (c + 1) * 128],
                            xc[:, c, kc * 128 : (kc + 1) * 128],
                            ident,
                        )
                    nc.scalar.copy(out=xt_f[:, kc, :], in_=pst[:, :])
                xt_b = xpool.tile([128, KC, TILE], BF16, tag="xtb")
                nc.vector.tensor_copy(out=xt_b[:, :, :], in_=xt_f[:, :, :])

                # ---- gating in f32: logitsT [E, TILE]
                lg_ps = ps_lg.tile([E, TILE], F32, tag="lgps")
                for kc in range(KC):
                    nc.tensor.matmul(
                        lg_ps[:, :],
                        wg_sb[:, kc * E : (kc + 1) * E],
                        xt_f[:, kc, :],
                        start=(kc == 0),
                        stop=(kc == KC - 1),
                    )
                lgT = spool.tile([E, TILE], F32, tag="lgT")
                nc.scalar.copy(out=lgT[:, :], in_=lg_ps[:, :])
                lg = spool.tile([128, NCH, E], F32, tag="lg")
                for c in range(NCH):
                    lg_t_ps = ps_lg.tile([128, E], F32, tag="lgtps")
                    nc.tensor.transpose(
                        lg_t_ps[:, :], lgT[:, c * 128 : (c + 1) * 128], ident[:E, :E]
                    )
                    nc.vector.tensor_copy(out=lg[:, c, :], in_=lg_t_ps[:, :])

                # ---- per-chunk top-2 gates
                gate = spool.tile([128, NCH, E], F32, tag="gate")
                for c in range(NCH):
                    lgc = lg[:, c, :]
                    mx = spool.tile([128, 8], F32, tag="mx")
                    nc.vector.max(out=mx[:, :], in_=lgc)
                    m1 = mx[:, 0:1]
                    m2 = mx[:, 1:2]
                    dd = spool.tile([128, 1], F32, tag="dd")
                    nc.vector.tensor_scalar(
                        out=dd[:, :], in0=m2, scalar1=m1, scalar2=None,
                        op0=ALU.subtract,
                    )
                    e2 = spool.tile([128, 1], F32, tag="e2")
                    nc.scalar.activation(e2[:, :], dd[:, :], ACTF.Exp)
                    den = spool.tile([128, 1], F32, tag="den")
                    nc.vector.tensor_scalar(
                        out=den[:, :], in0=e2[:, :], scalar1=1.0, scalar2=None,
                        op0=ALU.add,
                    )
                    rden = spool.tile([128, 1], F32, tag="rden")
                    nc.vector.reciprocal(rden[:, :], den[:, :])
                    g2 = spool.tile([128, 1], F32, tag="g2")
                    nc.vector.tensor_tensor(
                        out=g2[:, :], in0=e2[:, :], in1=rden[:, :], op=ALU.mult
                    )
                    mk1 = spool.tile([128, E], F32, tag="mk1")
                    nc.vector.tensor_scalar(
                        out=mk1[:, :], in0=lgc, scalar1=m1, scalar2=None,
                        op0=ALU.is_equal,
                    )
                    mk2 = spool.tile([128, E], F32, tag="mk2")
                    nc.vector.tensor_scalar(
                        out=mk2[:, :], in0=lgc, scalar1=m2, scalar2=None,
                        op0=ALU.is_equal,
                    )
                    t2 = spool.tile([128, E], F32, tag="t2")
                    nc.vector.tensor_scalar(
                        out=t2[:, :], in0=mk2[:, :], scalar1=g2[:, 0:1],
                        scalar2=None, op0=ALU.mult,
                    )
                    nc.vector.scalar_tensor_tensor(
                        out=gate[:, c, :], in0=mk1[:, :], scalar=rden[:, 0:1],
                        in1=t2[:, :], op0=ALU.mult, op1=ALU.add,
                    )
                    nc.vector.tensor_tensor(
                        out=gacc[:, :], in0=gacc[:, :], in1=gate[:, c, :], op=ALU.add
                    )
                    mk12 = spool.tile([128, E], F32, tag="mk12")
                    nc.vector.tensor_tensor(
                        out=mk12[:, :], in0=mk1[:, :], in1=mk2[:, :], op=ALU.add
                    )
                    nc.vector.tensor_tensor(
                        out=lacc[:, :], in0=lacc[:, :], in1=mk12[:, :], op=ALU.add
                    )

                # ---- dense experts
                ez_all = ezpool.tile([128, NCH, E, MO], BF16, tag="ez")
                s_all = spool.tile([128, NCH, E], F32, tag="sall")
                for e in range(E):
                    h_ps = ps_h.tile([128, TILE], F32, tag="hps")
                    for kc in range(KC):
                        nc.tensor.matmul(
                            h_ps[:, :],
                            w1_sb[:, (e * KC + kc) * H : (e * KC + kc + 1) * H],
                            xt_b[:, kc, :],
                            start=(kc == 0),
                            stop=(kc == KC - 1),
                        )
                    hT = wpool.tile([128, TILE], BF16, tag="hT")
                    if e % 2 == 0:
                        nc.scalar.activation(
                            hT[:, :], h_ps[:, :], ACTF.Relu, bias=b1_sb[:, e : e + 1]
                        )
                    else:
                        nc.vector.tensor_scalar(
                            out=hT[:, :], in0=h_ps[:, :],
                            scalar1=b1_sb[:, e : e + 1], scalar2=0.0,
                            op0=ALU.add, op1=ALU.max,
                        )
                    for c in range(NCH):
                        z_ps = ps_z.tile([128, MO], F32, tag="zps")
                        nc.tensor.matmul(
                            z_ps[:, :],
                            hT[:, c * 128 : (c + 1) * 128],
                            w2_sb[:, e * MO : (e + 1) * MO],
                            start=True,
                            stop=True,
                        )
                        nc.scalar.activation(
                            ez_all[:, c, e, :], z_ps[:, :], ACTF.Exp,
                            accum_out=s_all[:, c, e : e + 1],
                        )

                # ---- combine + head
                for c in range(NCH):
                    rs = spool.tile([128, E], F32, tag="rs")
                    nc.vector.reciprocal(rs[:, :], s_all[:, c, :])
                    coef = spool.tile([128, E], F32, tag="coef")
                    nc.vector.tensor_tensor(
                        out=coef[:, :], in0=gate[:, c, :], in1=rs[:, :], op=ALU.mult
                    )
                    y = wpool.tile([128, MO], BF16, tag="y")
                    nc.vector.tensor_scalar(
                        out=y[:, :], in0=ez_all[:, c, 0, :], scalar1=coef[:, 0:1],
                        scalar2=None, op0=ALU.mult,
                    )
                    for e in range(1, E):
                        nc.vector.scalar_tensor_tensor(
                            out=y[:, :], in0=ez_all[:, c, e, :],
                            scalar=coef[:, e : e + 1], in1=y[:, :],
                            op0=ALU.mult, op1=ALU.add,
                        )
                    scrap = wpool.tile([128, MO], BF16, tag="scrap")
                    for o in range(OUT):
                        nc.vector.scalar_tensor_tensor(
                            out=scrap[:, :], in0=y[:, :], scalar=1.0,
                            in1=wo_sb[:, o * MO : (o + 1) * MO],
                            op0=ALU.bypass, op1=ALU.mult,
                            accum_out=out_sb[:, (t * NCH + c) * OUT + o :
                                             (t * NCH + c) * OUT + o + 1],
                        )

            # ---- stats + output DMA
            st = cpool.tile([128, 2 * E], F32)
            nc.vector.tensor_copy(out=st[:, 0:E], in_=gacc[:, :])
            nc.vector.tensor_copy(out=st[:, E : 2 * E], in_=lacc[:, :])
            nc.sync.dma_start(st_d[:, :], st[:, :])
            # out rows (t*NCH+c)*128+p, col o  <- out_sb[p, (t*NCH+c)*OUT+o]
            out_view = out_d.rearrange("(g p) o -> p g o", p=128)
            nc.sync.dma_start(out_view, out_sb.rearrange("p (g o) -> p g o", o=OUT))
    if not nc.is_finalized():
        nc.finalize()
    return nc


def _prep_weight_maps(inputs: dict) -> dict:
    bf = ml_dtypes.bfloat16
    W1 = np.asarray(inputs["W1"], np.float32)      # [E, IN, H]
    W2 = np.asarray(inputs["W2"], np.float32)      # [E, H, MO]
    wg = np.asarray(inputs["w_gate"], np.float32)  # [IN, E]
    b1 = np.asarray(inputs["b1"], np.float32)      # [E, H]
    Wout = np.asarray(inputs["Wout"], np.float32)  # [MO, OUT]
    w1sb = np.ascontiguousarray(
        W1.reshape(E, KC, 128, H).transpose(2, 0, 1, 3)
    ).reshape(128, E * KC * H).astype(bf)
    w2sb = np.ascontiguousarray(W2.transpose(1, 0, 2)).reshape(128, E * MO).astype(bf)
    wgsb = np.ascontiguousarray(
        wg.reshape(KC, 128, E).transpose(1, 0, 2)
    ).reshape(128, KC * E).astype(np.float32)
    b1sb = np.ascontiguousarray(b1.T).astype(np.float32)  # [128, E]
    wosb = np.ascontiguousarray(
        np.broadcast_to(Wout.T.reshape(1, OUT * MO), (128, OUT * MO))
    ).astype(bf)
    return {"w1sb": w1sb, "w2sb": w2sb, "wgsb": wgsb, "b1sb": b1sb, "wosb": wosb}


def _run(inputs: dict, trace: bool = False):
    global _CACHED_NC
    if _CACHED_NC is None:
        _CACHED_NC = _build_nc()
    nc = _CACHED_NC
    x = np.ascontiguousarray(np.asarray(inputs["num_prop"], np.float32))
    wmaps = _prep_weight_maps(inputs)
    in_maps = [
        {"x": np.ascontiguousarray(x[i * R : (i + 1) * R]), **wmaps}
        for i in range(NCORES)
    ]
    res = run_bass_kernel_spmd(nc, in_maps, list(range(NCORES)), trace=trace)
    out = np.concatenate([res.results[i]["out"] for i in range(NCORES)], axis=0)
    bout = np.asarray(inputs["bout"], np.float32)
    out = out + bout[None, :]
    imp = np.zeros(E, np.float64)
    load = np.zeros(E, np.float64)
    for i in range(NCORES):
        st = np.asarray(res.results[i]["stats"], np.float64)
        imp += st[:, 0:E].sum(axis=0)
        load += st[:, E : 2 * E].sum(axis=0)
    cv2 = lambda v: v.var(ddof=1) / (v.mean() ** 2 + 1e-10)
    loss = np.float32((cv2(imp) + cv2(load)) * 1e-2)
    return (out.astype(np.float32), loss), res


def kernel(**inputs):
    (out, loss), _ = _run(inputs, trace=False)
    return out, loss
